# revision 28
# baseline (speedup 1.0000x reference)
"""Trainium2 Bass kernel for nn_BINLayer (binarized dense layer).

Computes out = sign(x) @ sign(W) + sign(bias) with sign(v >= 0) = +1 else -1
(forward value of the straight-through-estimator reference).

Strategy:
  - Data-parallel shard x over batch rows: 8 cores x 1024 rows each.
    W and bias are replicated; each core computes its full [1024, 4096]
    output slice, results are concatenated on the host.
  - The host ships x transposed ([D, B_shard]) so the contraction dim lands
    on SBUF partitions, and cast to bf16 (sign-exact: bf16 has the full f32
    exponent range, so sign(bf16(v)) == sign(v) for every input value).
  - On device: sign is computed on the Scalar engine (ACT Sign activation),
    emitting +-1 directly in fp8e4. The matmul runs on the Tensor engine in
    fp8 DoubleRow mode (2 fp8 weights per PE cell, contraction 256/matmul)
    with fp32 PSUM accumulation. Since all operands are exactly +-1 and row
    sums are integers <= 4097, the result is bit-exact vs float64.
  - Bias (sign-converted on device) is added during PSUM->SBUF eviction on
    the Vector engine, fused with the copy.
"""

import os
from contextlib import ExitStack

import numpy as np
import ml_dtypes

import concourse.bass as bass
from concourse import mybir
from concourse.bass_utils import run_bass_kernel_spmd

P = 128
D = 4096
B = 8192
N_CORES = 8
B_SHARD = B // N_CORES  # 1024
NFREE = 512  # psum free dim (one bank of fp32)

F32 = mybir.dt.float32
BF16 = mybir.dt.bfloat16
FP8 = mybir.dt.float8e4
U8D = mybir.dt.uint8

SIGN = mybir.ActivationFunctionType.Sign

# Stash of the most recent BassKernelResults (exec_time_ns etc) for test.py.
LAST_RESULTS = None


def build_nc(d=D, b_shard=B_SHARD, nfree=NFREE, use_fp8=True):
    """Build the per-core Bass program (raw bass: explicit engine streams and
    semaphores — the toolchain here rejects DMAs carrying >1 sync wait, so all
    waits are sequencer instructions). Every core runs this same program on
    its own batch shard.

    Input DMA is split across BOTH HWDGE rings (each ring serializes its
    transfers at ~435 GB/s): the SP ring carries x then W blocks 1+, the ACT
    ring carries W block 0 and the bias. Batches of up to 4 k-tiles keep the
    per-trigger issue cost amortized; the first two batches are half-size so
    the first matmul can start sooner.

    Sign work (bf16 -> +-1 fp8) is spread so no single engine paces the PE:
      ACT: x even tiles (Sign activation)
      DVE: x odd tiles + ALL W tiles + bias (u8 bit trick: take the high
           byte of each bf16, (b & 0x80) | 0x38 == +-1.0 in fp8e4), plus the
           PSUM->SBUF eviction fused with the bias add
    The fp8 W tiles live in a triple-buffered block buffer so block n+2's
    signs are produced while PE works block n+1. The bias is signed one
    512-wide slice at a time immediately before the evictions that need it.

    PE: fp8 DoubleRow matmuls (contraction 256/instr), fp32 PSUM accumulate.
    Block 0 runs k-major across all psum banks so each freshly signed k-pair
    immediately unlocks MT matmuls. POOL does the output DMAs (SWDGE).
    """
    KT = d // P        # contraction tiles of 128
    MT = b_shard // P  # output row tiles of 128
    NT = d // nfree    # output col blocks of nfree
    KK = KT // 2       # DoubleRow pairs
    XB = min(4, KT)    # max k-tiles per input DMA batch
    NB_O = 8           # out staging ring slots
    NGRP = NT * MT     # psum accumulation groups
    NWB = min(3, NT)   # wb block buffers
    cdt = FP8 if use_fp8 else BF16

    def make_batches(first_small):
        lst = []
        s = 0
        if first_small and KT >= XB and XB >= 4:
            lst = [(0, 2), (2, 2)]
            s = 4
        while s < KT:
            sz = min(XB, KT - s)
            lst.append((s, sz))
            s += sz
        return lst

    x_bat = make_batches(True)            # x batches (block-0 critical)
    w_bat0 = make_batches(True)           # W block-0 batches
    w_batn = make_batches(False)          # W block 1+ batches
    # global W batch list: (block, start_tile, size, end_global_tile)
    wbat = []
    for n in range(NT):
        for (st, sz) in (w_bat0 if n == 0 else w_batn):
            wbat.append((n, st, sz, n * KT + st + sz))
    NWD0 = len(w_bat0)  # W batches in block 0 (ACT ring); rest are SP's
    wmap = {}  # global tile j -> (batch_idx, sub)
    for bi, (n, st, sz, _) in enumerate(wbat):
        for u in range(sz):
            wmap[n * KT + st + u] = (bi, u)
    xmap = {}  # tile kt -> (batch_idx, sub)
    for bi, (st, sz) in enumerate(x_bat):
        for u in range(sz):
            xmap[st + u] = (bi, u)

    NB_X = min(4, len(x_bat))    # x staging ring slots (one batch each)
    NB_W = min(8, len(wbat))     # W staging ring slots (one batch each)

    nc = bass.Bass()
    xT = nc.declare_dram_parameter("xT", [d, b_shard], BF16, isOutput=False)
    W = nc.declare_dram_parameter("W", [d, d], U8D, isOutput=False)
    bias_b = nc.declare_dram_parameter("bias_b", [P, d], U8D, isOutput=False)
    out = nc.declare_dram_parameter("out", [b_shard, d], F32, isOutput=True)

    with ExitStack() as ctx:
        ent = ctx.enter_context
        xsb = ent(nc.sbuf_tensor("xsb", [P, NB_X, XB, b_shard], BF16))
        bx = ent(nc.sbuf_tensor("bx", [P, KT, b_shard], cdt))
        wsb = ent(nc.sbuf_tensor("wsb", [P, NB_W, XB, nfree], U8D))
        wb = ent(nc.sbuf_tensor("wb", [P, NWB, KT, nfree], cdt))
        bstage = ent(nc.sbuf_tensor("bstage", [P, d], U8D))
        bsb = ent(nc.sbuf_tensor("bsb", [P, d], cdt))
        osb = ent(nc.sbuf_tensor("osb", [P, NB_O, nfree], F32))
        warm = ent(nc.sbuf_tensor("warm", [P, 2, nfree], cdt))
        pst = [ent(nc.psum_tensor(f"pst{b}", [P, nfree], F32)) for b in range(8)]

        s_bd = ent(nc.semaphore("s_bd"))   # bias dma done (+16)
        s_bsg = ent(nc.semaphore("s_bsg"))  # bias slice signs (DVE, +1 each)
        # Sign sems are single-producer so "sem >= k" == "that engine's first
        # k tiles are done" (engine instruction streams complete in order).
        s_xs_a = ent(nc.semaphore("s_xs_a"))  # x even signs (ACT)
        s_xs_d = ent(nc.semaphore("s_xs_d"))  # x odd signs (DVE)
        s_wsd = ent(nc.semaphore("s_wsd"))    # W signs, all blocks (DVE)
        s_mm = ent(nc.semaphore("s_mm"))   # psum groups done (+1 each)
        s_ev = ent(nc.semaphore("s_ev"))   # evict+bias adds done (+1 each)
        # Per-slot DMA-completion sems: DMA completions across a shared sem
        # are unordered, so "sem >= 16*(k+1)" would not mean "DMA k landed".
        # One sem per ring slot with at most one DMA in flight per slot makes
        # the thresholds sound.
        s_xd = [ent(nc.semaphore(f"s_xd{i}")) for i in range(NB_X)]
        s_wd = [ent(nc.semaphore(f"s_wd{i}")) for i in range(NB_W)]
        s_od = [ent(nc.semaphore(f"s_od{i}")) for i in range(NB_O)]
        s_warm = ent(nc.semaphore("s_warm"))
        all_sems = [s_bd, s_bsg, s_xs_a, s_xs_d, s_wsd, s_mm, s_ev, s_warm,
                    *s_xd, *s_wd, *s_od]

        def wslice(n):
            return slice(n * nfree, (n + 1) * nfree)

        def wait_xsign(eng, kt):
            """Wait until x tile kt has been sign-converted."""
            if kt % 2 == 0:
                eng.wait_ge(s_xs_a, kt // 2 + 1)
            else:
                eng.wait_ge(s_xs_d, kt // 2 + 1)

        U8 = mybir.dt.uint8

        def dve_sign(vector, dst_ap, src_ap):
            """+-1 fp8e4 sign via bit ops: (hi_byte(bf16) & 0x80) | 0x38."""
            return vector.tensor_scalar(
                out=dst_ap.bitcast(U8),
                in0=src_ap.bitcast(U8)[:, 1::2],
                scalar1=0x80,
                scalar2=0x38,
                op0=mybir.AluOpType.bitwise_and,
                op1=mybir.AluOpType.bitwise_or,
            )

        def dve_sign_u8(vector, dst_ap, src_ap):
            """Same bit trick, but the source is already the bf16 high byte
            (the DRAM W tensor ships as uint8 to halve its DMA traffic)."""
            return vector.tensor_scalar(
                out=dst_ap.bitcast(U8),
                in0=src_ap,
                scalar1=0x80,
                scalar2=0x38,
                op0=mybir.AluOpType.bitwise_and,
                op1=mybir.AluOpType.bitwise_or,
            )

        def batched(dram_slice):
            """[S*P, C] DRAM slice -> [P, S, C] AP (row s*P + p -> [p, s])."""
            return dram_slice.rearrange("(s p) c -> p s c", p=P)

        def w_dma(eng, bi):
            n, wst, wsz, _ = wbat[bi]
            if bi >= NB_W:
                eng.wait_ge(s_wsd, wbat[bi - NB_W][3])
            eng.dma_start(
                out=wsb[:, bi % NB_W, :wsz],
                in_=batched(W[wst * P:(wst + wsz) * P, wslice(n)]),
            ).then_inc(s_wd[bi % NB_W], 16)

        with nc.Block() as block:

            @block.sync
            def _(sync):
                # x batches, then W blocks 1+ (W block 0 and the bias go down
                # the ACT HWDGE ring in parallel with this stream)
                for i, (st, sz) in enumerate(x_bat):
                    if i >= NB_X:
                        # slot free once both parities of batch i-NB_X signed
                        pst_, psz = x_bat[i - NB_X]
                        wait_xsign(sync, pst_ + psz - 1)
                        if psz > 1:
                            wait_xsign(sync, pst_ + psz - 2)
                    sync.dma_start(
                        out=xsb[:, i % NB_X, :sz],
                        in_=batched(xT[st * P:(st + sz) * P, :]),
                    ).then_inc(s_xd[i % NB_X], 16)
                for bi in range(NWD0, len(wbat)):
                    w_dma(sync, bi)
                # last block's out-DMAs ride this (by now idle) HWDGE ring, so
                # the kernel doesn't end on a slow SWDGE drain of POOL's queue
                for g in range((NT - 1) * MT, NGRP):
                    n, m = g // MT, g % MT
                    sync.wait_ge(s_ev, g + 1)
                    sync.dma_start(
                        out=out[m * P:(m + 1) * P, wslice(n)],
                        in_=osb[:, g % NB_O, :],
                    ).then_inc(s_od[g % NB_O], 16)
                for i in range(NB_O):
                    n_dmas = len([g for g in range((NT - 1) * MT, NGRP)
                                  if g % NB_O == i])
                    base = len([g for g in range((NT - 1) * MT)
                                if g % NB_O == i])
                    if n_dmas:
                        sync.wait_ge(s_od[i], 16 * (base + n_dmas))

            @block.scalar
            def _(scalar):
                # Interleave the ACT-ring DMA triggers (W block 0, bias) with
                # the x even-tile signs: a HWDGE trigger occupies the issuing
                # sequencer for its whole transfer, so issuing all of them
                # up-front would stall the first sign (and the first matmul)
                # behind ~14us of transfers.
                evens = list(range(0, KT, 2))
                ops = []
                for bi in range(NWD0):
                    ops.append(("w", bi))
                    if bi >= 1 and evens:
                        ops.append(("x", evens.pop(0)))
                ops += [("x", kt) for kt in evens]
                ops.append(("b", 0))
                for kind, v in ops:
                    if kind == "w":
                        w_dma(scalar, v)
                    elif kind == "b":
                        scalar.dma_start(
                            out=bstage[:, :], in_=bias_b[:, :]
                        ).then_inc(s_bd, 16)
                    else:
                        bi, sub = xmap[v]
                        scalar.wait_ge(s_xd[bi % NB_X], 16 * (bi // NB_X + 1))
                        scalar.activation(
                            bx[:, v, :], xsb[:, bi % NB_X, sub, :], SIGN
                        ).then_inc(s_xs_a, 1)

            @block.tensor
            def _(tensor):
                # Warmup: the PE clock gate (HAM) needs ~3.4us of sustained
                # activity to lift the idle 4/8 throttle. The real matmuls
                # only start once the first signed k-pair is ready (~8us after
                # engines start), so burn that window on throwaway matmuls —
                # they read whatever is in SBUF and their PSUM contribution is
                # discarded by block 0's start=True.
                tensor.wait_ge(s_warm, 1)
                for _ in range(28):
                    tensor.matmul(
                        pst[0][:, :],
                        warm[:, :, 0:P],
                        warm[:, :, :],
                        start=True,
                        stop=True,
                        perf_mode=mybir.MatmulPerfMode.DoubleRow,
                    )
                # Block 0 runs k-major across all MT psum banks for the
                # prefix (each freshly signed k-pair immediately unlocks MT
                # matmuls, so the PE is never starved behind the serial
                # prologue sign chain), then m-major for the last TK pairs so
                # the groups complete staggered and evictions can start early.
                TK = max(1, min(4, KK // 2))
                for kk in range(KK - TK):
                    wait_xsign(tensor, 2 * kk)
                    wait_xsign(tensor, 2 * kk + 1)
                    tensor.wait_ge(s_wsd, 2 * kk + 2)
                    for m in range(MT):
                        tensor.matmul(
                            pst[m % 8][:, :],
                            bx[:, 2 * kk:2 * kk + 2, m * P:(m + 1) * P],
                            wb[:, 0, 2 * kk:2 * kk + 2, :],
                            start=(kk == 0),
                            stop=False,
                            perf_mode=mybir.MatmulPerfMode.DoubleRow,
                        )
                for kk in range(KK - TK, KK):
                    wait_xsign(tensor, 2 * kk)
                    wait_xsign(tensor, 2 * kk + 1)
                    tensor.wait_ge(s_wsd, 2 * kk + 2)
                for m in range(MT):
                    for kk in range(KK - TK, KK):
                        mm = tensor.matmul(
                            pst[m % 8][:, :],
                            bx[:, 2 * kk:2 * kk + 2, m * P:(m + 1) * P],
                            wb[:, 0, 2 * kk:2 * kk + 2, :],
                            start=False,
                            stop=(kk == KK - 1),
                            perf_mode=mybir.MatmulPerfMode.DoubleRow,
                        )
                    mm.then_inc(s_mm, 1)
                # Blocks 1+: m-major, one bank per group; the first m-tile of
                # each block is k-gated so a lagging sign stream degrades
                # smoothly instead of stalling the whole block.
                for n in range(1, NT):
                    for m in range(MT):
                        g = n * MT + m
                        if g >= 8:
                            tensor.wait_ge(s_ev, g - 7)
                        for kk in range(KK):
                            if m == 0:
                                tensor.wait_ge(s_wsd, n * KT + 2 * kk + 2)
                            mm = tensor.matmul(
                                pst[g % 8][:, :],
                                bx[:, 2 * kk:2 * kk + 2, m * P:(m + 1) * P],
                                wb[:, n % NWB, 2 * kk:2 * kk + 2, :],
                                start=(kk == 0),
                                stop=(kk == KK - 1),
                                perf_mode=mybir.MatmulPerfMode.DoubleRow,
                            )
                        mm.then_inc(s_mm, 1)

            @block.vector
            def _(vector):
                def wsign(j):
                    """Sign W tile j (global index) into its wb slot."""
                    n, kt = j // KT, j % KT
                    bi, sub = wmap[j]
                    vector.wait_ge(s_wd[bi % NB_W], 16 * (bi // NB_W + 1))
                    dve_sign_u8(
                        vector,
                        wb[:, n % NWB, kt, :],
                        wsb[:, bi % NB_W, sub, :],
                    ).then_inc(s_wsd, 1)

                # Block-0 prologue: x odd-tile signs interleaved with W
                # block-0 signs in exactly PE consumption order
                for kk in range(KK):
                    kt = 2 * kk + 1
                    bi, sub = xmap[kt]
                    vector.wait_ge(s_xd[bi % NB_X], 16 * (bi // NB_X + 1))
                    dve_sign(
                        vector, bx[:, kt, :], xsb[:, bi % NB_X, sub, :]
                    ).then_inc(s_xs_d, 1)
                    wsign(2 * kk)
                    wsign(2 * kk + 1)
                # W block-1 signs
                for kt in range(KT if NT > 1 else 0):
                    wsign(KT + kt)
                # steady state: sign this block's bias slice, trail the
                # block's evictions, then sign W block n+2 (its wb slot was
                # freed by block n-1, which these evictions' s_mm waits have
                # already implied)
                for n in range(NT):
                    if n == 0:
                        vector.wait_ge(s_bd, 16)
                    dve_sign_u8(
                        vector, bsb[:, wslice(n)], bstage[:, wslice(n)]
                    ).then_inc(s_bsg, 1)
                    for m in range(MT):
                        g = n * MT + m
                        vector.wait_ge(s_mm, g + 1)
                        vector.wait_ge(s_bsg, n + 1)
                        if g >= NB_O:
                            vector.wait_ge(s_od[g % NB_O], 16 * (g // NB_O))
                        vector.tensor_add(
                            osb[:, g % NB_O, :], pst[g % 8][:, :],
                            bsb[:, wslice(n)],
                        ).then_inc(s_ev, 1)
                    if n + 2 < NT:
                        for kt in range(KT):
                            wsign((n + 2) * KT + kt)

            @block.gpsimd
            def _(gpsimd):
                gpsimd.memset(warm[:, :, :], 0.0).then_inc(s_warm, 1)
                for g in range((NT - 1) * MT):
                    n, m = g // MT, g % MT
                    gpsimd.wait_ge(s_ev, g + 1)
                    gpsimd.dma_start(
                        out=out[m * P:(m + 1) * P, wslice(n)],
                        in_=osb[:, g % NB_O, :],
                    ).then_inc(s_od[g % NB_O], 16)
                # drain own DMAs before the end-of-block barrier
                for i in range(NB_O):
                    n_dmas = len([g for g in range((NT - 1) * MT)
                                  if g % NB_O == i])
                    if n_dmas:
                        gpsimd.wait_ge(s_od[i], 16 * n_dmas)

        # Block exit emitted drain + all-engine barrier: every stream is done.
        # Zero the semaphores (spread over the engines so the clears run in
        # parallel) so a re-execution of the loaded NEFF starts clean.
        clear_engines = [nc.sync, nc.scalar, nc.vector, nc.tensor, nc.gpsimd]
        for i, s in enumerate(all_sems):
            clear_engines[i % len(clear_engines)].sem_clear(s)

    return nc


def _prep_inputs(x, W, bias):
    """Host-side shard/layout prep: transpose x, cast to bf16 (sign-exact),
    replicate bias across the 128 partitions."""
    xT = np.ascontiguousarray(np.asarray(x).astype(ml_dtypes.bfloat16).T)
    Wb16 = np.ascontiguousarray(np.asarray(W).astype(ml_dtypes.bfloat16))
    Wb = np.ascontiguousarray((Wb16.view(np.uint16) >> 8).astype(np.uint8))
    bb16 = np.asarray(bias).astype(ml_dtypes.bfloat16)
    bu8 = (bb16.view(np.uint16) >> 8).astype(np.uint8)
    bias_b = np.ascontiguousarray(np.broadcast_to(bu8[None, :], (P, D)))
    in_maps = []
    for c in range(N_CORES):
        in_maps.append(
            {
                "xT": np.ascontiguousarray(xT[:, c * B_SHARD:(c + 1) * B_SHARD]),
                "W": Wb,
                "bias_b": bias_b,
            }
        )
    return in_maps


def kernel(x, W, bias):
    global LAST_RESULTS
    in_maps = _prep_inputs(x, W, bias)
    nc = build_nc()
    res = run_bass_kernel_spmd(
        nc,
        in_maps,
        core_ids=list(range(N_CORES)),
        trace=bool(int(os.environ.get("KBASS_TRACE", "0"))),
    )
    LAST_RESULTS = res
    out = np.concatenate([r["out"] for r in res.results], axis=0)
    return np.ascontiguousarray(out.astype(np.float32))


# revision 29
# speedup vs baseline: 1.1768x; 1.1768x over previous
"""Trainium2 Bass kernel for nn_BINLayer (binarized dense layer).

Computes out = sign(x) @ sign(W) + sign(bias) with sign(v >= 0) = +1 else -1
(forward value of the straight-through-estimator reference).

Strategy:
  - Data-parallel shard x over batch rows: 8 cores x 1024 rows each.
    W and bias are replicated; each core computes its full [1024, 4096]
    output slice, results are concatenated on the host.
  - The host ships x transposed ([D, B_shard]) so the contraction dim lands
    on SBUF partitions, and cast to bf16 (sign-exact: bf16 has the full f32
    exponent range, so sign(bf16(v)) == sign(v) for every input value).
  - On device: sign is computed on the Scalar engine (ACT Sign activation),
    emitting +-1 directly in fp8e4. The matmul runs on the Tensor engine in
    fp8 DoubleRow mode (2 fp8 weights per PE cell, contraction 256/matmul)
    with fp32 PSUM accumulation. Since all operands are exactly +-1 and row
    sums are integers <= 4097, the result is bit-exact vs float64.
  - Bias (sign-converted on device) is added during PSUM->SBUF eviction on
    the Vector engine, fused with the copy.
"""

import os
from contextlib import ExitStack

import numpy as np
import ml_dtypes

import concourse.bass as bass
from concourse import mybir
from concourse.bass_utils import run_bass_kernel_spmd

P = 128
D = 4096
B = 8192
N_CORES = 8
B_SHARD = B // N_CORES  # 1024
NFREE = 512  # psum free dim (one bank of fp32)

F32 = mybir.dt.float32
BF16 = mybir.dt.bfloat16
FP8 = mybir.dt.float8e4
U8D = mybir.dt.uint8

SIGN = mybir.ActivationFunctionType.Sign

# Stash of the most recent BassKernelResults (exec_time_ns etc) for test.py.
LAST_RESULTS = None


def build_nc(d=D, b_shard=B_SHARD, nfree=NFREE, use_fp8=True):
    """Build the per-core Bass program (raw bass: explicit engine streams and
    semaphores — the toolchain here rejects DMAs carrying >1 sync wait, so all
    waits are sequencer instructions). Every core runs this same program on
    its own batch shard.

    Input DMA is split across BOTH HWDGE rings (each ring serializes its
    transfers at ~435 GB/s): the SP ring carries x then W blocks 1+, the ACT
    ring carries W block 0 and the bias. Batches of up to 4 k-tiles keep the
    per-trigger issue cost amortized; the first two batches are half-size so
    the first matmul can start sooner.

    Sign work (bf16 -> +-1 fp8) is spread so no single engine paces the PE:
      ACT: x even tiles (Sign activation)
      DVE: x odd tiles + ALL W tiles + bias (u8 bit trick: take the high
           byte of each bf16, (b & 0x80) | 0x38 == +-1.0 in fp8e4), plus the
           PSUM->SBUF eviction fused with the bias add
    The fp8 W tiles live in a triple-buffered block buffer so block n+2's
    signs are produced while PE works block n+1. The bias is signed one
    512-wide slice at a time immediately before the evictions that need it.

    PE: fp8 DoubleRow matmuls (contraction 256/instr), fp32 PSUM accumulate.
    Block 0 runs k-major across all psum banks so each freshly signed k-pair
    immediately unlocks MT matmuls. POOL does the output DMAs (SWDGE).
    """
    KT = d // P        # contraction tiles of 128
    MT = b_shard // P  # output row tiles of 128
    NT = d // nfree    # output col blocks of nfree
    KK = KT // 2       # DoubleRow pairs
    XB = min(4, KT)    # max k-tiles per input DMA batch
    NB_O = 8           # out staging ring slots
    NGRP = NT * MT     # psum accumulation groups
    NWB = min(3, NT)   # wb block buffers
    cdt = FP8 if use_fp8 else BF16

    def make_batches(first_small):
        lst = []
        s = 0
        if first_small and KT >= XB and XB >= 4:
            lst = [(0, 2), (2, 2)]
            s = 4
        while s < KT:
            sz = min(XB, KT - s)
            lst.append((s, sz))
            s += sz
        return lst

    x_bat = make_batches(True)            # x batches (block-0 critical)
    w_bat0 = make_batches(True)           # W block-0 batches
    w_batn = make_batches(False)          # W block 1+ batches
    # global W batch list: (block, start_tile, size, end_global_tile)
    wbat = []
    for n in range(NT):
        for (st, sz) in (w_bat0 if n == 0 else w_batn):
            wbat.append((n, st, sz, n * KT + st + sz))
    NWD0 = len(w_bat0)  # W batches in block 0 (ACT ring); rest are SP's
    wmap = {}  # global tile j -> (batch_idx, sub)
    for bi, (n, st, sz, _) in enumerate(wbat):
        for u in range(sz):
            wmap[n * KT + st + u] = (bi, u)
    xmap = {}  # tile kt -> (batch_idx, sub)
    for bi, (st, sz) in enumerate(x_bat):
        for u in range(sz):
            xmap[st + u] = (bi, u)

    NB_X = min(4, len(x_bat))    # x staging ring slots (one batch each)
    NB_W = min(8, len(wbat))     # W staging ring slots (one batch each)

    nc = bass.Bass()
    xT = nc.declare_dram_parameter("xT", [d, b_shard], BF16, isOutput=False)
    W = nc.declare_dram_parameter("W", [d, d], U8D, isOutput=False)
    bias_b = nc.declare_dram_parameter("bias_b", [P, d], BF16, isOutput=False)
    out = nc.declare_dram_parameter("out", [b_shard, d], F32, isOutput=True)

    with ExitStack() as ctx:
        ent = ctx.enter_context
        xsb = ent(nc.sbuf_tensor("xsb", [P, NB_X, XB, b_shard], BF16))
        bx = ent(nc.sbuf_tensor("bx", [P, KT, b_shard], cdt))
        wsb = ent(nc.sbuf_tensor("wsb", [P, NB_W, XB, nfree], U8D))
        wb = ent(nc.sbuf_tensor("wb", [P, NWB, KT, nfree], cdt))
        bstage = ent(nc.sbuf_tensor("bstage", [P, d], BF16))
        bsb = ent(nc.sbuf_tensor("bsb", [P, d], cdt))
        osb = ent(nc.sbuf_tensor("osb", [P, NB_O, nfree], F32))
        warm = ent(nc.sbuf_tensor("warm", [P, 2, nfree], cdt))
        pst = [ent(nc.psum_tensor(f"pst{b}", [P, nfree], F32)) for b in range(8)]

        s_bd = ent(nc.semaphore("s_bd"))   # bias dma done (+16)
        s_bsg = ent(nc.semaphore("s_bsg"))  # bias slice signs (DVE, +1 each)
        # Sign sems are single-producer so "sem >= k" == "that engine's first
        # k tiles are done" (engine instruction streams complete in order).
        s_xs_a = ent(nc.semaphore("s_xs_a"))  # x even signs (ACT)
        s_xs_d = ent(nc.semaphore("s_xs_d"))  # x odd signs (DVE)
        s_wsd = ent(nc.semaphore("s_wsd"))    # W signs, all blocks (DVE)
        s_mm = ent(nc.semaphore("s_mm"))   # psum groups done (+1 each)
        s_ev = ent(nc.semaphore("s_ev"))   # evict+bias adds done (+1 each)
        # Per-slot DMA-completion sems: DMA completions across a shared sem
        # are unordered, so "sem >= 16*(k+1)" would not mean "DMA k landed".
        # One sem per ring slot with at most one DMA in flight per slot makes
        # the thresholds sound.
        s_xd = [ent(nc.semaphore(f"s_xd{i}")) for i in range(NB_X)]
        s_wd = [ent(nc.semaphore(f"s_wd{i}")) for i in range(NB_W)]
        s_od = [ent(nc.semaphore(f"s_od{i}")) for i in range(NB_O)]
        s_warm = ent(nc.semaphore("s_warm"))
        all_sems = [s_bd, s_bsg, s_xs_a, s_xs_d, s_wsd, s_mm, s_ev, s_warm,
                    *s_xd, *s_wd, *s_od]

        def wslice(n):
            return slice(n * nfree, (n + 1) * nfree)

        def wait_xsign(eng, kt):
            """Wait until x tile kt has been sign-converted."""
            if kt % 2 == 0:
                eng.wait_ge(s_xs_a, kt // 2 + 1)
            else:
                eng.wait_ge(s_xs_d, kt // 2 + 1)

        U8 = mybir.dt.uint8

        def dve_sign(vector, dst_ap, src_ap):
            """+-1 fp8e4 sign via bit ops: (hi_byte(bf16) & 0x80) | 0x38."""
            return vector.tensor_scalar(
                out=dst_ap.bitcast(U8),
                in0=src_ap.bitcast(U8)[:, 1::2],
                scalar1=0x80,
                scalar2=0x38,
                op0=mybir.AluOpType.bitwise_and,
                op1=mybir.AluOpType.bitwise_or,
            )

        def dve_sign_u8(vector, dst_ap, src_ap):
            """Same bit trick, but the source is already the bf16 high byte
            (the DRAM W tensor ships as uint8 to halve its DMA traffic)."""
            return vector.tensor_scalar(
                out=dst_ap.bitcast(U8),
                in0=src_ap,
                scalar1=0x80,
                scalar2=0x38,
                op0=mybir.AluOpType.bitwise_and,
                op1=mybir.AluOpType.bitwise_or,
            )

        def batched(dram_slice):
            """[S*P, C] DRAM slice -> [P, S, C] AP (row s*P + p -> [p, s])."""
            return dram_slice.rearrange("(s p) c -> p s c", p=P)

        def w_dma(eng, bi):
            n, wst, wsz, _ = wbat[bi]
            if bi >= NB_W:
                eng.wait_ge(s_wsd, wbat[bi - NB_W][3])
            eng.dma_start(
                out=wsb[:, bi % NB_W, :wsz],
                in_=batched(W[wst * P:(wst + wsz) * P, wslice(n)]),
            ).then_inc(s_wd[bi % NB_W], 16)

        with nc.Block() as block:

            @block.sync
            def _(sync):
                # x batches, then W blocks 1+ (W block 0 and the bias go down
                # the ACT HWDGE ring in parallel with this stream)
                for i, (st, sz) in enumerate(x_bat):
                    if i >= NB_X:
                        # slot free once both parities of batch i-NB_X signed
                        pst_, psz = x_bat[i - NB_X]
                        wait_xsign(sync, pst_ + psz - 1)
                        if psz > 1:
                            wait_xsign(sync, pst_ + psz - 2)
                    sync.dma_start(
                        out=xsb[:, i % NB_X, :sz],
                        in_=batched(xT[st * P:(st + sz) * P, :]),
                    ).then_inc(s_xd[i % NB_X], 16)
                for bi in range(NWD0, len(wbat)):
                    w_dma(sync, bi)
                # last block's out-DMAs ride this (by now idle) HWDGE ring, so
                # the kernel doesn't end on a slow SWDGE drain of POOL's queue
                for g in range((NT - 1) * MT, NGRP):
                    n, m = g // MT, g % MT
                    sync.wait_ge(s_ev, g + 1)
                    sync.dma_start(
                        out=out[m * P:(m + 1) * P, wslice(n)],
                        in_=osb[:, g % NB_O, :],
                    ).then_inc(s_od[g % NB_O], 16)
                for i in range(NB_O):
                    n_dmas = len([g for g in range((NT - 1) * MT, NGRP)
                                  if g % NB_O == i])
                    base = len([g for g in range((NT - 1) * MT)
                                if g % NB_O == i])
                    if n_dmas:
                        sync.wait_ge(s_od[i], 16 * (base + n_dmas))

            @block.scalar
            def _(scalar):
                # Interleave the ACT-ring DMA triggers (W block 0, bias) with
                # the x even-tile signs: a HWDGE trigger occupies the issuing
                # sequencer for its whole transfer, so issuing all of them
                # up-front would stall the first sign (and the first matmul)
                # behind ~14us of transfers.
                evens = list(range(0, KT, 2))
                ops = []
                for bi in range(NWD0):
                    ops.append(("w", bi))
                    if bi >= 1 and evens:
                        ops.append(("x", evens.pop(0)))
                    if bi == NWD0 - 2:
                        ops.append(("b", 0))
                ops += [("x", kt) for kt in evens]
                for kind, v in ops:
                    if kind == "w":
                        w_dma(scalar, v)
                    elif kind == "b":
                        scalar.dma_start(
                            out=bstage[:, :], in_=bias_b[:, :]
                        ).then_inc(s_bd, 16)
                    else:
                        bi, sub = xmap[v]
                        scalar.wait_ge(s_xd[bi % NB_X], 16 * (bi // NB_X + 1))
                        scalar.activation(
                            bx[:, v, :], xsb[:, bi % NB_X, sub, :], SIGN
                        ).then_inc(s_xs_a, 1)

            @block.tensor
            def _(tensor):
                # Warmup: the PE clock gate (HAM) needs ~3.4us of sustained
                # activity to lift the idle 4/8 throttle. The real matmuls
                # only start once the first signed k-pair is ready (~8us after
                # engines start), so burn that window on throwaway matmuls —
                # they read whatever is in SBUF and their PSUM contribution is
                # discarded by block 0's start=True.
                tensor.wait_ge(s_warm, 1)
                for _ in range(28):
                    tensor.matmul(
                        pst[0][:, :],
                        warm[:, :, 0:P],
                        warm[:, :, :],
                        start=True,
                        stop=True,
                        perf_mode=mybir.MatmulPerfMode.DoubleRow,
                    )
                # Block 0 runs k-major across all MT psum banks for the
                # prefix (each freshly signed k-pair immediately unlocks MT
                # matmuls, so the PE is never starved behind the serial
                # prologue sign chain), then m-major for the last TK pairs so
                # the groups complete staggered and evictions can start early.
                TK = max(1, min(4, KK // 2))
                for kk in range(KK - TK):
                    wait_xsign(tensor, 2 * kk)
                    wait_xsign(tensor, 2 * kk + 1)
                    tensor.wait_ge(s_wsd, 2 * kk + 2)
                    for m in range(MT):
                        tensor.matmul(
                            pst[m % 8][:, :],
                            bx[:, 2 * kk:2 * kk + 2, m * P:(m + 1) * P],
                            wb[:, 0, 2 * kk:2 * kk + 2, :],
                            start=(kk == 0),
                            stop=False,
                            perf_mode=mybir.MatmulPerfMode.DoubleRow,
                        )
                for kk in range(KK - TK, KK):
                    wait_xsign(tensor, 2 * kk)
                    wait_xsign(tensor, 2 * kk + 1)
                    tensor.wait_ge(s_wsd, 2 * kk + 2)
                for m in range(MT):
                    for kk in range(KK - TK, KK):
                        mm = tensor.matmul(
                            pst[m % 8][:, :],
                            bx[:, 2 * kk:2 * kk + 2, m * P:(m + 1) * P],
                            wb[:, 0, 2 * kk:2 * kk + 2, :],
                            start=False,
                            stop=(kk == KK - 1),
                            perf_mode=mybir.MatmulPerfMode.DoubleRow,
                        )
                    mm.then_inc(s_mm, 1)
                # Blocks 1+: m-major, one bank per group; the first m-tile of
                # each block is k-gated so a lagging sign stream degrades
                # smoothly instead of stalling the whole block.
                for n in range(1, NT):
                    for m in range(MT):
                        g = n * MT + m
                        if g >= 8:
                            tensor.wait_ge(s_ev, g - 7)
                        for kk in range(KK):
                            if m == 0:
                                tensor.wait_ge(s_wsd, n * KT + 2 * kk + 2)
                            mm = tensor.matmul(
                                pst[g % 8][:, :],
                                bx[:, 2 * kk:2 * kk + 2, m * P:(m + 1) * P],
                                wb[:, n % NWB, 2 * kk:2 * kk + 2, :],
                                start=(kk == 0),
                                stop=(kk == KK - 1),
                                perf_mode=mybir.MatmulPerfMode.DoubleRow,
                            )
                        mm.then_inc(s_mm, 1)

            @block.vector
            def _(vector):
                def wsign(j):
                    """Sign W tile j (global index) into its wb slot."""
                    n, kt = j // KT, j % KT
                    bi, sub = wmap[j]
                    vector.wait_ge(s_wd[bi % NB_W], 16 * (bi // NB_W + 1))
                    dve_sign_u8(
                        vector,
                        wb[:, n % NWB, kt, :],
                        wsb[:, bi % NB_W, sub, :],
                    ).then_inc(s_wsd, 1)

                # Block-0 prologue: x odd-tile signs interleaved with W
                # block-0 signs in exactly PE consumption order
                for kk in range(KK):
                    kt = 2 * kk + 1
                    bi, sub = xmap[kt]
                    vector.wait_ge(s_xd[bi % NB_X], 16 * (bi // NB_X + 1))
                    dve_sign(
                        vector, bx[:, kt, :], xsb[:, bi % NB_X, sub, :]
                    ).then_inc(s_xs_d, 1)
                    wsign(2 * kk)
                    wsign(2 * kk + 1)
                # W block-1 signs
                for kt in range(KT if NT > 1 else 0):
                    wsign(KT + kt)
                # steady state: sign this block's bias slice, trail the
                # block's evictions, then sign W block n+2 (its wb slot was
                # freed by block n-1, which these evictions' s_mm waits have
                # already implied)
                for n in range(NT):
                    if n == 0:
                        vector.wait_ge(s_bd, 16)
                    dve_sign(
                        vector, bsb[:, wslice(n)], bstage[:, wslice(n)]
                    ).then_inc(s_bsg, 1)
                    for m in range(MT):
                        g = n * MT + m
                        vector.wait_ge(s_mm, g + 1)
                        vector.wait_ge(s_bsg, n + 1)
                        if g >= NB_O:
                            vector.wait_ge(s_od[g % NB_O], 16 * (g // NB_O))
                        vector.tensor_add(
                            osb[:, g % NB_O, :], pst[g % 8][:, :],
                            bsb[:, wslice(n)],
                        ).then_inc(s_ev, 1)
                    if n + 2 < NT:
                        for kt in range(KT):
                            wsign((n + 2) * KT + kt)

            @block.gpsimd
            def _(gpsimd):
                gpsimd.memset(warm[:, :, :], 0.0).then_inc(s_warm, 1)
                for g in range((NT - 1) * MT):
                    n, m = g // MT, g % MT
                    gpsimd.wait_ge(s_ev, g + 1)
                    gpsimd.dma_start(
                        out=out[m * P:(m + 1) * P, wslice(n)],
                        in_=osb[:, g % NB_O, :],
                    ).then_inc(s_od[g % NB_O], 16)
                # drain own DMAs before the end-of-block barrier
                for i in range(NB_O):
                    n_dmas = len([g for g in range((NT - 1) * MT)
                                  if g % NB_O == i])
                    if n_dmas:
                        gpsimd.wait_ge(s_od[i], 16 * n_dmas)

        # Block exit emitted drain + all-engine barrier: every stream is done.
        # Zero the semaphores (spread over the engines so the clears run in
        # parallel) so a re-execution of the loaded NEFF starts clean.
        clear_engines = [nc.sync, nc.scalar, nc.vector, nc.tensor, nc.gpsimd]
        for i, s in enumerate(all_sems):
            clear_engines[i % len(clear_engines)].sem_clear(s)

    return nc


def _prep_inputs(x, W, bias):
    """Host-side shard/layout prep: transpose x, cast to bf16 (sign-exact),
    replicate bias across the 128 partitions."""
    xT = np.ascontiguousarray(np.asarray(x).astype(ml_dtypes.bfloat16).T)
    Wb16 = np.ascontiguousarray(np.asarray(W).astype(ml_dtypes.bfloat16))
    Wb = np.ascontiguousarray((Wb16.view(np.uint16) >> 8).astype(np.uint8))
    bias_b = np.ascontiguousarray(
        np.broadcast_to(
            np.asarray(bias).astype(ml_dtypes.bfloat16)[None, :], (P, D)
        )
    )
    in_maps = []
    for c in range(N_CORES):
        in_maps.append(
            {
                "xT": np.ascontiguousarray(xT[:, c * B_SHARD:(c + 1) * B_SHARD]),
                "W": Wb,
                "bias_b": bias_b,
            }
        )
    return in_maps


def kernel(x, W, bias):
    global LAST_RESULTS
    in_maps = _prep_inputs(x, W, bias)
    nc = build_nc()
    res = run_bass_kernel_spmd(
        nc,
        in_maps,
        core_ids=list(range(N_CORES)),
        trace=bool(int(os.environ.get("KBASS_TRACE", "0"))),
    )
    LAST_RESULTS = res
    out = np.concatenate([r["out"] for r in res.results], axis=0)
    return np.ascontiguousarray(out.astype(np.float32))
